# revision 8
# baseline (speedup 1.0000x reference)
"""Capsule-routing kernel (einsum bni,nkdi,nk->bkd + squash) on 8 trn2 cores.

Sharding: over the contraction axis n (2048 -> 256 per core).  Each core
reads only its slice of x and W -- every input byte is read exactly once
machine-wide.  Each core emits a partial s[b,(k,d)] over its n-slice; the
host sums the 8 partials and applies the tiny squash nonlinearity.

v2 changes vs the 40.2us baseline (trace-driven):
  - W is shipped as int8 with a per-(n,k) host-computed scale a_w folded
    into the softmax(R) multiplier: wb = int8(W) * bf16(Rs*a_w).  Halves
    the dominant HBM stream (4MB -> 2MB/core); measured end-to-end rel
    err ~0.8% (gate 2e-2).
  - ALL input DMAs ride ONE HWDGE queue (sync) in explicit arrival
    order: rs, x0a, W0c0, x0b, W0c1, W0c2, x1, W1c0..2.  The old 2-queue
    layout round-robined x behind W, landing x0 at t=18.5us and stalling
    the first matmul.  Single-queue FIFO lands x0a ~4us in.
  - Rs is uploaded un-broadcast [n, k] (16KB not 256KB); the scale op
    broadcasts over (i, d) with stride-0 AP dims.
  - PE warm-up: ~36 dummy 128x128 matmuls on the identity tile right
    after the preamble keep the PE HAM busy so it unthrottles to 2.4GHz
    (K=8/8) before the first real matmul (trace showed the first ~14
    real matmuls ran at 1.2GHz).
  - Output DMAs go SWDGE (gpsimd, per-DMA semaphores) because the 10
    input DMAs exhaust the 8 HWDGE DMAHW lanes and a reused lane would
    add a second sem-wait (illegal in this walrus build).
  - acc0 (B rows 0..127) finishes 3 matmuls early (tail reordered
    h0,h0,h1,h1) so its PSUM evac + output DMA overlap acc1's tail.
  - Tile's kernel-sem range narrowed (teardown probe): the NEFF epilogue
    clears every semaphore one instruction at a time (~6.6us!); if the
    clear range follows the declared range this shrinks it.

The walrus build in this container accepts at most ONE sync-wait per
instruction.  Consequences handled here:
  - tiny DVE "toucher" ops absorb the rs/x DMA completions into DVE
    program order (so matmuls and scale ops carry at most one wait)
  - W-chunk scale ops carry their chunk's DMA wait directly (their other
    operand rs is already DVE-ordered)
  - input DMAs may reuse DMAHW lanes (their only wait); output DMAs are
    SWDGE so their single wait is the evac dependency
  - Tile's multi-wait kernel-tail drain is monkeypatched into a chain of
    single-wait drains
"""

import os
import sys

import numpy as np

if "/opt/trn_rl_repo" not in sys.path:
    sys.path.insert(0, "/opt/trn_rl_repo")

import bass_rust as _bass_rust
import concourse.bass as bass
import concourse.mybir as mybir
import ml_dtypes
from concourse.bass_utils import run_bass_kernel_spmd
from concourse.masks import make_identity
from concourse.tile import TileContext

NCORES = 8
B, N, I = 256, 2048, 16
K, D = 32, 16
NL = N // NCORES  # 256 n-values per core
KD = K * D  # 512
F_W = I * K * D  # 8192   (i-major W layout)
F_X = I * B  # 4096      (x^T layout: [n, i, B])
EPS = 1e-7

FP32 = mybir.dt.float32
BF16 = mybir.dt.bfloat16
INT8 = mybir.dt.int8
NPBF16 = ml_dtypes.bfloat16

N_WARM_MM = 40  # dummy matmuls to unthrottle the PE HAM before real work

# W chunk boundaries in units of i (per 128-partition tile); last chunk
# small so the final sem-wait + scale + matmul tail is short
WCHUNKS = [(0, 4), (4, 9), (9, 14), (14, 16)]

# Narrow the Tile kernel-sem range (teardown-storm probe; see docstring).
if not getattr(bass, "_sem_range_patched", False):
    bass.get_kernel_semaphore_range = lambda: range(150, 212)
    bass._sem_range_patched = True

# Teardown-storm probe #2: cap walrus's own semaphore budget.  The NEFF
# epilogue clears semaphores one instruction at a time (~6.3us measured);
# if the clear range follows the compiler budget this shrinks it.
import concourse.bass_utils as _bu

if not getattr(_bu, "_walrus_args_patched", False):
    _orig_gwa = _bu.get_walrus_args

    def _gwa(arch, tmpdir, *, dve_root=None):
        return _orig_gwa(arch, tmpdir, dve_root=dve_root) + ["--max-sem-num=64"]

    _bu.get_walrus_args = _gwa
    _bu._walrus_args_patched = True

# Split Tile's multi-wait kernel-tail drain into a chain of single-wait
# drains (program order on the sync sequencer makes the chain equivalent).
if not getattr(TileContext, "_split_drain_patched", False):

    def _split_drain_and_barrier(self, tick_clock, wait_clock):
        gc = tick_clock.global_clock
        vals = list(gc)
        for j, v in enumerate(vals):
            if v > 0:
                sub = [0] * len(vals)
                sub[j] = v
                d = self.nc.sync.drain()
                wait_clock.add_sem_waits(
                    d.ins,
                    _bass_rust.ScopedClock({None: _bass_rust.VectorClock(sub)}),
                )
        self.nc.all_engine_barrier()
        assert self.sems is not None
        popped = self.nc._tile_sem_poison_stack.pop()
        assert popped is self._sem_poison
        self.nc.clear_and_free_semaphores(list(self.sems.allocated().values()))

    TileContext._drain_and_barrier = _split_drain_and_barrier
    TileContext._split_drain_patched = True


def build_bass() -> bass.Bass:
    nc = bass.Bass()
    x_d = nc.dram_tensor("xs", [NL, F_X], BF16, kind="ExternalInput")
    w_d = nc.dram_tensor("ws", [NL, F_W], INT8, kind="ExternalInput")
    r_d = nc.dram_tensor("rs", [NL, K], BF16, kind="ExternalInput")
    o_d = nc.dram_tensor("out", [B, KD], FP32, kind="ExternalOutput")

    with TileContext(nc) as tc:
        with (
            tc.tile_pool(name="big", bufs=1) as big,
            tc.tile_pool(name="ps_warm", bufs=1, space="PSUM") as ps_warm,
            tc.tile_pool(name="ps_acc", bufs=1, space="PSUM") as ps_acc,
        ):
            rs_kd = big.tile([128, 2 * K], BF16, tag="rs_kd")
            xb = [big.tile([128, F_X], BF16, tag=f"x{t}", name=f"x{t}") for t in range(2)]
            # W lands as bf16: the SWDGE cast-DMA converts the int8 wire
            # format during the transfer (HBM reads stay 1B/elem)
            wc = [big.tile([128, F_W], BF16, tag=f"w{t}", name=f"w{t}") for t in range(2)]
            wb = [big.tile([128, F_W], BF16, tag=f"wb{t}", name=f"wb{t}") for t in range(2)]

            # ---- input DMAs.  rs + x-tile-0 ride the sync HWDGE queue;
            # W (cast int8 -> bf16 in the DMA datapath) and x-tile-1 ride
            # the gpsimd SWDGE queue (per-DMA semaphores), FIFO-ordered
            # W0, x1, W1 so tile-1 data lands mid-stream, not last. ----
            nc.sync.dma_start(
                out=rs_kd[:], in_=r_d.rearrange("(t p) k -> p t k", t=2)
            )
            nc.sync.dma_start(
                out=xb[0][:, : 8 * B], in_=x_d[0:128, : 8 * B]
            )
            nc.sync.dma_start(
                out=xb[0][:, 8 * B :], in_=x_d[0:128, 8 * B :]
            )

            # ---- PE warm-up: identity (gpsimd) -> transpose absorbs the
            # gpsimd dep into PE order -> dummy matmuls keep HAM busy ----
            identb = big.tile([128, 128], BF16, tag="identb")
            make_identity(nc, identb)

            def dma_w(t, ci):
                i0, i1 = WCHUNKS[ci]
                nc.gpsimd.dma_start(
                    out=wc[t][:, i0 * KD : i1 * KD],
                    in_=w_d[t * 128 : (t + 1) * 128, i0 * KD : i1 * KD],
                )

            for ci in range(len(WCHUNKS)):
                dma_w(0, ci)
            nc.gpsimd.dma_start(out=xb[1][:], in_=x_d[128:256, :])
            for ci in range(len(WCHUNKS)):
                dma_w(1, ci)
            warm_tp = ps_warm.tile([128, 128], BF16, tag="warmtp")
            nc.tensor.transpose(warm_tp[:], identb[:], identb[:])
            warm_mm = ps_warm.tile([128, 128], FP32, tag="warmmm")
            for _ in range(N_WARM_MM):
                nc.tensor.matmul(
                    warm_mm[:], identb[:], identb[:], start=True, stop=True
                )

            # ---- DVE pipeline in arrival order: touchers (absorb rs/x
            # DMA completions) interleaved with per-chunk scale ops ----
            def touch(name, src):
                tt = big.tile([128, 1], BF16, tag=f"touch_{name}")
                nc.vector.tensor_copy(tt[:], src)

            def scale(t, ci):
                i0, i1 = WCHUNKS[ci]
                ni = i1 - i0
                sl_in = wc[t][:, i0 * KD : i1 * KD].rearrange(
                    "p (i k d) -> p i k d", k=K, d=D
                )
                sl_out = wb[t][:, i0 * KD : i1 * KD].rearrange(
                    "p (i k d) -> p i k d", k=K, d=D
                )
                r_sl = rs_kd[:, t * K : (t + 1) * K]
                r_b = bass.AP(
                    tensor=r_sl.tensor,
                    offset=r_sl.offset,
                    ap=[r_sl.ap[0], [0, ni], [1, K], [0, D]],
                )
                nc.vector.tensor_mul(sl_out, sl_in, r_b)

            touch("rs", rs_kd[:, 0:1])
            touch("x0a", xb[0][:, 0:1])
            scale(0, 0)
            touch("x0b", xb[0][:, 8 * B : 8 * B + 1])
            scale(0, 1)
            scale(0, 2)
            scale(0, 3)
            touch("x1", xb[1][:, 0:1])
            scale(1, 0)
            scale(1, 1)
            scale(1, 2)
            scale(1, 3)

            # ---- main matmuls ----
            # acc_h[b, (k d)] += xb[t][:, (i, h-half)]^T @ wb[t][:, i-slice].
            # Tail reordered h0,h0,h1,h1 so acc0 finalizes early and its
            # evac + output DMA overlap acc1's last matmuls.
            accs = [
                ps_acc.tile([128, KD], FP32, tag=f"acc{h}", name=f"acc{h}")
                for h in range(2)
            ]

            def mm(t, i, h, start, stop):
                rhs = wb[t][:, i * KD : (i + 1) * KD]
                lhsT = xb[t][:, i * B + h * 128 : i * B + (h + 1) * 128]
                nc.tensor.matmul(accs[h][:], lhsT, rhs, start=start, stop=stop)

            for t in range(2):
                for i in range(I):
                    if t == 1 and i >= I - 2:
                        continue
                    first = t == 0 and i == 0
                    mm(t, i, 0, first, False)
                    mm(t, i, 1, first, False)
            mm(1, I - 2, 0, False, False)
            mm(1, I - 1, 0, False, True)
            mm(1, I - 2, 1, False, False)
            mm(1, I - 1, 1, False, True)

            # ---- output: PSUM -> SBUF on DVE, HWDGE out on fresh DMAHW
            # lanes (sync carried only 3 input DMAs), single evac wait ----
            o_sb = big.tile([128, 2 * KD], FP32, tag="osb")
            for h in range(2):
                nc.vector.tensor_copy(o_sb[:, h * KD : (h + 1) * KD], accs[h][:])
                nc.sync.dma_start(
                    out=o_d[h * 128 : (h + 1) * 128, :],
                    in_=o_sb[:, h * KD : (h + 1) * KD],
                )

    return nc


_CACHE: dict = {}

# test.py sets these for profiling; harness never touches them.
LAST_RESULTS = None


def _trace_kwargs():
    if os.environ.get("BASS_KERNEL_TRACE") == "1":
        cores = os.environ.get("BASS_KERNEL_TRACE_CORES", "0")
        return dict(trace=True, trace_cores=[int(c) for c in cores.split(",")])
    return {}


def kernel(x: np.ndarray, W: np.ndarray, R: np.ndarray) -> np.ndarray:
    global LAST_RESULTS
    x = np.asarray(x, dtype=np.float32)
    W = np.asarray(W, dtype=np.float32)
    R = np.asarray(R, dtype=np.float32)

    # softmax over n (65K elements -- host)
    Rm = R.max(axis=0, keepdims=True)
    e = np.exp(R - Rm)
    Rs = (e / e.sum(axis=0, keepdims=True)).astype(np.float32)

    # per-(n,k) int8 quantization of W; the dequant scale a_w rides the
    # softmax multiplier (wb = int8(W) * bf16(Rs*a_w) on device)
    Wi = np.ascontiguousarray(W.transpose(0, 3, 1, 2))  # [N, I, K, D]
    a_w = np.abs(W).max(axis=(2, 3)) / 127.0 + 1e-30  # [N, K]
    W8 = np.clip(np.rint(Wi / a_w[:, None, :, None]), -127, 127).astype(np.int8)
    Wp = W8.reshape(N, F_W)
    Rp = np.ascontiguousarray(Rs * a_w).astype(NPBF16)  # [N, K]
    Xp = np.ascontiguousarray(x.transpose(1, 2, 0)).reshape(N, F_X).astype(NPBF16)

    in_maps = []
    for c in range(NCORES):
        sl = slice(c * NL, (c + 1) * NL)
        in_maps.append({"xs": Xp[sl], "ws": Wp[sl], "rs": Rp[sl]})

    if "nc" not in _CACHE:
        _CACHE["nc"] = build_bass()
    nc = _CACHE["nc"]

    res = run_bass_kernel_spmd(
        nc, in_maps, core_ids=list(range(NCORES)), **_trace_kwargs()
    )
    LAST_RESULTS = res

    s = np.zeros((B, KD), np.float32)
    for r in res.results:
        s += r["out"]
    s = s.reshape(B, K, D)
    sq = np.sum(np.square(s), axis=-1, keepdims=True) + EPS
    v = (np.sqrt(sq) / (1.0 + sq)) * s
    return v.astype(np.float32)


if __name__ == "__main__":
    rng = np.random.default_rng(0)
    x = rng.standard_normal((B, N, I), dtype=np.float32)
    W = (rng.standard_normal((N, K, D, I), dtype=np.float32) * 0.05).astype(np.float32)
    R = rng.standard_normal((N, K), dtype=np.float32)
    out = kernel(x, W, R)
    print("out", out.shape, out.dtype, float(np.abs(out).mean()))
